# revision 27
# baseline (speedup 1.0000x reference)
"""Trainium2 Bass kernel for nn_CBERTProtoConcat (retrieval_knn).

Computation (reference):
    hq = query @ W1[:, :D].T            [Q, 2D]
    hs = support @ W1[:, D:].T          [S, 2D]
    scores[q,s] = sum_d W2[d] * relu(hq[q,d] + hs[s,d] + b1[d])    [Q, S]
    agg = segment-mean of scores by support label                  [Q, C]
    loss = -mean(log_softmax(agg)[q, target_q]);  correct = argmax(agg) == target

Strategy: shard Q across 8 cores (16 queries each), replicate everything else
(collective-free).  Per core, layout: d' (=2D) on partitions, s on free.
  - hqT [2D, 16] and hsT [2D, S] via PE matmuls (f32r), pipelined with W1T DMA.
  - per-query chains: ACT/DVE/Pool compute tmp = relu(hsT_m + hqT[:,q]) [128, 512],
    PE accumulates scores_q[1, 512] += W2_m.T @ tmp in PSUM (f32r, 12 steps).
  - epilogue: transpose scores, one-hot matmul -> agg [16, C], softmax/loss/argmax.
b2 is skipped: it shifts all logits uniformly (log_softmax/argmax invariant).
"""

import numpy as np

NCORES = 8
Q, S, D = 128, 512, 768
D2 = 2 * D            # 1536
C = 64                # num classes
QL = Q // NCORES      # 16 queries per core
KT = D // 128         # 6 contraction tiles per slice
MT = D2 // 128        # 12 d' tiles
QB = 4                # query block size in phase 2

# engine assignment pattern for tmp production (7 DVE, 3 ACT, 2 Pool per 12)
TMP_ENG = ["D", "D", "A", "D", "D", "A", "D", "D", "A", "D", "D", "A"]

# packed input layouts
QSW_QT0 = 0                      # qt_k at [.., 16k:16k+16]
QSW_SUP0 = QL * KT               # 96;  supt_k at [96+512k : 96+512(k+1)]
QSW_W2 = QSW_SUP0 + S * KT       # 3168; w2 cols [3168:3180]
QSW_W = QSW_W2 + MT              # 3180

MISC_B1 = 0                      # [128, 0:12]
MISC_T1H = MT                    # [0:16, 12:76]
MISC_TREV = MISC_T1H + C         # [0:16, 76:77]
MISC_REVI = MISC_TREV + 1        # [0:16, 77:141]
MISC_ID = MISC_REVI + C          # [0:16, 141:157]
MISC_MHOT = MISC_ID + QL         # [128, 157:157+256]
MISC_W = MISC_MHOT + 4 * C       # 413

LAST_EXEC_NS = None

_CACHE = {}


def _build_program(reps=1):
    from contextlib import ExitStack

    import concourse.tile as tile
    from concourse import bacc, mybir

    f32 = mybir.dt.float32
    f32r = mybir.dt.float32r
    AF = mybir.ActivationFunctionType
    OP = mybir.AluOpType

    nc = bacc.Bacc(
        "TRN2",
        target_bir_lowering=False,
        debug=False,
        enable_asserts=False,
        num_devices=NCORES,
    )

    # ---- I/O ----
    # w1t[m] is [128, KT*2*128]: slice1 chunk k at [:, 128k:...], slice2 at [:, (KT+k)*128:...]
    w1t_d = nc.dram_tensor("w1t", [MT, 128, 2 * KT * 128], f32r, kind="ExternalInput")
    qtw2_d = nc.dram_tensor("qtw2", [128, QL * KT + MT], f32r, kind="ExternalInput")
    supt_d = nc.dram_tensor("supt", [KT, 128, S], f32r, kind="ExternalInput")
    misc_d = nc.dram_tensor("misc", [128, MISC_W], f32, kind="ExternalInput")

    out_d = nc.dram_tensor("out4", [QL, 4], f32, kind="ExternalOutput")

    with tile.TileContext(nc) as tc, ExitStack() as ctx:
        const = ctx.enter_context(tc.tile_pool(name="const", bufs=1))
        w1pool = ctx.enter_context(tc.tile_pool(name="w1", bufs=3))
        hstp = ctx.enter_context(tc.tile_pool(name="hst", bufs=1))
        hqtp = ctx.enter_context(tc.tile_pool(name="hqt", bufs=1))
        tmpp = ctx.enter_context(tc.tile_pool(name="tmp", bufs=12))
        srowp = ctx.enter_context(tc.tile_pool(name="srow", bufs=3))
        epi = ctx.enter_context(tc.tile_pool(name="epi", bufs=1))
        ps_sc = ctx.enter_context(tc.tile_pool(name="ps_sc", bufs=QB, space="PSUM"))
        ps_h = ctx.enter_context(tc.tile_pool(name="ps_h", bufs=2, space="PSUM"))
        ps_q = ctx.enter_context(tc.tile_pool(name="ps_q", bufs=1, space="PSUM"))
        ps_ep = ctx.enter_context(tc.tile_pool(name="ps_ep", bufs=1, space="PSUM"))

        dma = nc.sync.dma_start

        misc = const.tile([128, MISC_W], f32, tag="misc")
        dma(misc[:], misc_d.ap())
        b1sb = misc[:, MISC_B1:MISC_B1 + MT]
        t1h = misc[0:QL, MISC_T1H:MISC_T1H + C]
        trev = misc[0:QL, MISC_TREV:MISC_TREV + 1]
        revi = misc[0:QL, MISC_REVI:MISC_REVI + C]
        ident = misc[0:QL, MISC_ID:MISC_ID + QL]
        mhot = [misc[:, MISC_MHOT + c4 * C: MISC_MHOT + (c4 + 1) * C] for c4 in range(4)]

        # preload the ACT Exp table off the critical path
        warm = const.tile([1, 2], f32, tag="warm")
        nc.scalar.activation(warm[0:1, 0:1], misc[0:1, 0:1], AF.Exp)

        for rep_i in range(reps):
            qtw2 = const.tile([128, QL * KT + MT], f32r, tag="qtw2")
            dma(qtw2[:], qtw2_d.ap())
            qt = [qtw2[:, QL * k:QL * (k + 1)] for k in range(KT)]
            w2sb = qtw2[:, QL * KT:QL * KT + MT]
            supt = []
            for k in range(KT):
                sup_k = const.tile([128, S], f32r, tag=f"sup{k}")
                supt.append(sup_k[:])
            dma(supt[0], supt_d.ap()[0])

            # chain step helper: tmp = relu(hst[m] + hqT col) on DVE/ACT, then
            # PE accumulates the W2-weighted reduction into the chain's PSUM row
            def chain_step(chains, qb, m, qi):
                q = qb + qi
                col = hqtb[m][:, q:q + 1]
                tmp = tmpp.tile([128, S], f32r, tag="tmp", name=f"tmp{qb}_{m}_{qi}")
                if TMP_ENG[(m * QB + qi) % 12] == "D":
                    nc.vector.tensor_scalar(
                        tmp[:], hst[m][:], col, 0.0, op0=OP.add, op1=OP.max
                    )
                else:
                    nc.scalar.activation(tmp[:], hst[m][:], AF.Relu, bias=col)
                nc.tensor.matmul(
                    chains[qi][0:1, :],
                    lhsT=w2sb[:, m:m + 1],
                    rhs=tmp[:],
                    start=(m == 0),
                    stop=(m == MT - 1),
                )

            def finish_block(chains, qb, bs):
                # stage the block's score rows and transpose into sct columns
                scrow = srowp.tile([QB, S], f32, tag="scrow", name=f"scrow{qb}")
                for qi in range(bs):
                    srow = srowp.tile([1, S], f32, tag="srow", name=f"srow{qb}_{qi}")
                    nc.scalar.copy(srow[:], chains[qi][0:1, :])
                    dma(scrow[qi:qi + 1, :], srow[:])
                for c4 in range(4):
                    nc.tensor.transpose(
                        sct_ps[:, c4 * QL + qb:c4 * QL + qb + bs],
                        scrow[0:bs, c4 * 128:(c4 + 1) * 128],
                        ident[0:bs, 0:bs],
                    )

            def open_block(bs):
                chains = []
                for qi in range(bs):
                    ch = ps_sc.tile([1, S], f32, tag="qs", name=f"qs{qi}")
                    chains.append(ch)
                return chains

            # ---- phase 1 + block-0 chains, pipelined with per-chunk W1T DMA ----
            epbank = ps_ep.tile([128, 128], f32, tag="epbank")
            sct_ps = epbank[:, 0:4 * QL]
            hqtb = []
            hst = []
            for m in range(MT):
                w1 = w1pool.tile([128, 2 * KT * 128], f32r, tag="w1")
                dma(w1[:], w1t_d.ap()[m])
                if m == 0:
                    for k in range(1, KT):
                        dma(supt[k], supt_d.ap()[k])

                ph = ps_h.tile([128, S], f32, tag="ph")
                for k in range(KT):
                    nc.tensor.matmul(
                        ph[:],
                        lhsT=w1[:, (KT + k) * 128:(KT + k + 1) * 128],
                        rhs=supt[k],
                        start=(k == 0),
                        stop=(k == KT - 1),
                    )
                hs_m = hstp.tile([128, S], f32, tag=f"hst{m}")
                nc.vector.tensor_copy(hs_m[:], ph[:])
                hst.append(hs_m)

                pq = ps_q.tile([128, QL], f32, tag="pq")
                for k in range(KT):
                    nc.tensor.matmul(
                        pq[:],
                        lhsT=w1[:, k * 128:(k + 1) * 128],
                        rhs=qt[k],
                        start=(k == 0),
                        stop=(k == KT - 1),
                    )
                hq_m = hqtp.tile([128, QL], f32, tag=f"hqtb{m}")
                nc.vector.tensor_scalar(
                    hq_m[:], pq[:], b1sb[:, m:m + 1], None, op0=OP.add
                )
                hqtb.append(hq_m)

            # ---- phase 2: query blocks ----
            for qb, bs in ((0, 4), (4, 4), (8, 4), (12, 2), (14, 2)):
                chains = open_block(bs)
                for m in range(MT):
                    for qi in range(bs):
                        chain_step(chains, qb, m, qi)
                finish_block(chains, qb, bs)

            # ---- epilogue ----
            sct_sb = epi.tile([128, 4 * QL], f32, tag="sct_sb")
            nc.vector.tensor_copy(sct_sb[:], sct_ps[:])

            agg_ps = epbank[0:QL, 4 * QL:4 * QL + C]
            for c4 in range(4):
                nc.tensor.matmul(
                    agg_ps[:],
                    lhsT=sct_sb[:, c4 * QL:(c4 + 1) * QL],
                    rhs=mhot[c4],
                    start=(c4 == 0),
                    stop=(c4 == 3),
                )
            agg_sb = epi.tile([QL, C], f32, tag="agg_sb")
            nc.vector.tensor_copy(agg_sb[:], agg_ps[:])

            # out4 columns: [ssum, mneg, zt, corr]; host computes
            # loss_q = log(ssum) - mneg - zt
            out4 = epi.tile([QL, 4], f32, tag="out4")

            mneg = out4[:, 1:2]
            nc.vector.tensor_reduce(mneg, agg_sb[:], mybir.AxisListType.X,
                                    OP.max, negate=True)
            mpos = epi.tile([QL, 1], f32, tag="mpos")
            nc.vector.tensor_scalar(mpos[:], mneg, -1.0, None, op0=OP.mult)

            ex = epi.tile([QL, C], f32, tag="ex")
            nc.scalar.activation(ex[:], agg_sb[:], AF.Exp, bias=mneg,
                                 accum_out=out4[:, 0:1])

            zt_t = epi.tile([QL, C], f32, tag="zt_t")
            nc.vector.tensor_tensor(zt_t[:], t1h, agg_sb[:], op=OP.mult)
            nc.vector.tensor_reduce(out4[:, 2:3], zt_t[:],
                                    mybir.AxisListType.X, OP.add)

            eq = epi.tile([QL, C], f32, tag="eq")
            nc.vector.tensor_scalar(eq[:], agg_sb[:], mpos[:, 0:1], None,
                                    op0=OP.is_equal)
            ev = epi.tile([QL, C], f32, tag="ev")
            nc.vector.tensor_tensor(ev[:], eq[:], revi, op=OP.mult)
            am = epi.tile([QL, 1], f32, tag="am")
            nc.vector.tensor_reduce(am[:], ev[:], mybir.AxisListType.X, OP.max)
            nc.vector.tensor_scalar(out4[:, 3:4], am[:], trev, None,
                                    op0=OP.is_equal)
            dma(out_d.ap(), out4[:])

    nc.compile()
    return nc


def _prep_inputs(query_reps, support_reps, W1, b1, W2, support_labels,
                 query_target_ids):
    """Host-side data layout: transposes, packing, one-hot metadata."""
    q = np.ascontiguousarray(np.asarray(query_reps, dtype=np.float32))
    sup = np.ascontiguousarray(np.asarray(support_reps, dtype=np.float32))
    W1 = np.asarray(W1, dtype=np.float32)
    b1 = np.asarray(b1, dtype=np.float32)
    W2 = np.asarray(W2, dtype=np.float32)
    labels = np.asarray(support_labels).astype(np.int64)
    tgt = np.asarray(query_target_ids).astype(np.int64)

    W1T = np.ascontiguousarray(W1.T)  # [2D, 2D]; W1T[d, d'] = W1[d', d]
    w1t = np.empty((MT, 128, 2 * KT * 128), np.float32)
    for m in range(MT):
        cols = W1T[:, m * 128:(m + 1) * 128]          # [2D, 128]
        for k in range(KT):
            w1t[m, :, k * 128:(k + 1) * 128] = cols[k * 128:(k + 1) * 128, :]
            w1t[m, :, (KT + k) * 128:(KT + k + 1) * 128] = \
                cols[D + k * 128:D + (k + 1) * 128, :]

    supT = sup.T  # [D, S]

    misc = np.zeros((128, MISC_W), np.float32)
    misc[:, MISC_B1:MISC_B1 + MT] = b1.reshape(MT, 128).T
    misc[0:QL, MISC_REVI:MISC_REVI + C] = np.broadcast_to(
        (C - 1 - np.arange(C)).astype(np.float32), (QL, C))
    misc[0:QL, MISC_ID:MISC_ID + QL] = np.eye(QL, dtype=np.float32)
    cnt = np.bincount(labels, minlength=C).astype(np.float32)
    mhot = np.zeros((S, C), np.float32)
    mhot[np.arange(S), labels] = (1.0 / np.maximum(cnt, 1.0))[labels]
    for c4 in range(4):
        misc[:, MISC_MHOT + c4 * C:MISC_MHOT + (c4 + 1) * C] = \
            mhot[c4 * 128:(c4 + 1) * 128]

    supt_in = np.ascontiguousarray(supT.reshape(KT, 128, S))

    in_maps = []
    for i in range(NCORES):
        qi = q[i * QL:(i + 1) * QL]                  # [QL, D]
        qT = qi.T                                    # [D, QL]
        qtw2 = np.zeros((128, QL * KT + MT), np.float32)
        for k in range(KT):
            qtw2[:, QL * k:QL * (k + 1)] = qT[k * 128:(k + 1) * 128]
        qtw2[:, QL * KT:QL * KT + MT] = W2[0].reshape(MT, 128).T
        ti = tgt[i * QL:(i + 1) * QL]
        mi = misc.copy()
        t1h = np.zeros((QL, C), np.float32)
        t1h[np.arange(QL), ti] = 1.0
        mi[0:QL, MISC_T1H:MISC_T1H + C] = t1h
        mi[0:QL, MISC_TREV:MISC_TREV + 1] = \
            (C - 1 - ti).astype(np.float32).reshape(QL, 1)
        in_maps.append({"w1t": w1t, "qtw2": qtw2, "supt": supt_in, "misc": mi})
    return in_maps


def get_program(reps=1):
    if reps not in _CACHE:
        _CACHE[reps] = _build_program(reps=reps)
    return _CACHE[reps]


def run_on_device(nc, in_maps):
    from concourse.bass_utils import run_bass_kernel_spmd
    return run_bass_kernel_spmd(nc, in_maps, core_ids=list(range(NCORES)),
                                trace=False)


def kernel(query_reps, support_reps, W1, b1, W2, b2, support_labels,
           query_target_ids, num_classes):
    global LAST_EXEC_NS
    assert int(num_classes) == C

    nc = get_program(reps=1)
    in_maps = _prep_inputs(query_reps, support_reps, W1, b1, W2,
                           support_labels, query_target_ids)
    res = run_on_device(nc, in_maps)
    LAST_EXEC_NS = res.exec_time_ns

    out = np.concatenate([res.results[i]["out4"] for i in range(NCORES)], axis=0)
    ssum, mneg, zt, corrf = out[:, 0], out[:, 1], out[:, 2], out[:, 3]
    loss_all = np.log(ssum) - mneg - zt
    loss = np.float32(np.mean(loss_all))
    correct = corrf > 0.5
    return loss, correct


# revision 28
# speedup vs baseline: 810.3076x; 810.3076x over previous
"""Trainium2 Bass kernel for nn_CBERTProtoConcat (retrieval_knn).

Computation (reference):
    hq = query @ W1[:, :D].T            [Q, 2D]
    hs = support @ W1[:, D:].T          [S, 2D]
    scores[q,s] = sum_d W2[d] * relu(hq[q,d] + hs[s,d] + b1[d])    [Q, S]
    agg = segment-mean of scores by support label                  [Q, C]
    loss = -mean(log_softmax(agg)[q, target_q]);  correct = argmax(agg) == target

Strategy: shard Q across 8 cores (16 queries each), replicate everything else
(collective-free).  Per core, layout: d' (=2D) on partitions, s on free.
  - hqT [2D, 16] and hsT [2D, S] via PE matmuls (f32r @ 1 cyc/row), pipelined
    against the per-chunk W1T DMA stream.
  - per-query reduction chains, in blocks of <=4 (PSUM bank limited): DVE/ACT
    compute tmp = relu(hsT_m + hqT[:,q]) [128, 512] (dual-op tensor_scalar on
    DVE / biased Relu activation on ACT, ~2:1 split), and PE accumulates
    scores_q[1, 512] += W2_m.T @ tmp in PSUM over the 12 m-steps (f32r).
  - per-block: score rows staged via partition-0 SBUF + DMA, transposed by PE
    into scoresT columns; then one-hot matmul -> agg [16, C] (count-normalized
    one-hot built on host from the labels), exp/sums on device.
  - outputs per core: [softmax-sum, -max, target-logit, correct] x 16; the
    host finishes with log(ssum) and the mean over 128 queries.
b2 is skipped: it shifts all logits uniformly (log_softmax/argmax invariant).
Cost-model (TimelineSim) estimate: ~88 us end-to-end per core; engine busy:
PE 62, DVE 53, ACT 51 us.
"""

import numpy as np

NCORES = 8
Q, S, D = 128, 512, 768
D2 = 2 * D            # 1536
C = 64                # num classes
QL = Q // NCORES      # 16 queries per core
KT = D // 128         # 6 contraction tiles per slice
MT = D2 // 128        # 12 d' tiles
QB = 4                # query block size in phase 2

# engine assignment pattern for tmp production (7 DVE, 3 ACT, 2 Pool per 12)
TMP_ENG = ["D", "D", "A", "D", "D", "A", "D", "D", "A", "D", "D", "A"]

# packed input layouts
QSW_QT0 = 0                      # qt_k at [.., 16k:16k+16]
QSW_SUP0 = QL * KT               # 96;  supt_k at [96+512k : 96+512(k+1)]
QSW_W2 = QSW_SUP0 + S * KT       # 3168; w2 cols [3168:3180]
QSW_W = QSW_W2 + MT              # 3180

MISC_B1 = 0                      # [128, 0:12]
MISC_T1H = MT                    # [0:16, 12:76]
MISC_TREV = MISC_T1H + C         # [0:16, 76:77]
MISC_REVI = MISC_TREV + 1        # [0:16, 77:141]
MISC_ID = MISC_REVI + C          # [0:16, 141:157]
MISC_MHOT = MISC_ID + QL         # [128, 157:157+256]
MISC_W = MISC_MHOT + 4 * C       # 413

LAST_EXEC_NS = None

_CACHE = {}


def _build_program(reps=1):
    from contextlib import ExitStack

    import concourse.tile as tile
    from concourse import bacc, mybir

    f32 = mybir.dt.float32
    f32r = mybir.dt.float32r
    AF = mybir.ActivationFunctionType
    OP = mybir.AluOpType

    nc = bacc.Bacc(
        "TRN2",
        target_bir_lowering=False,
        debug=False,
        enable_asserts=False,
        num_devices=NCORES,
    )

    # ---- I/O ----
    # w1t[m] is [128, KT*2*128]: slice1 chunk k at [:, 128k:...], slice2 at [:, (KT+k)*128:...]
    w1t_d = nc.dram_tensor("w1t", [MT, 128, 2 * KT * 128], f32r, kind="ExternalInput")
    qtw2_d = nc.dram_tensor("qtw2", [128, QL * KT + MT], f32r, kind="ExternalInput")
    supt_d = nc.dram_tensor("supt", [KT, 128, S], f32r, kind="ExternalInput")
    misc_d = nc.dram_tensor("misc", [128, MISC_W], f32, kind="ExternalInput")

    out_d = nc.dram_tensor("out4", [QL, 4], f32, kind="ExternalOutput")

    with tile.TileContext(nc) as tc, ExitStack() as ctx:
        const = ctx.enter_context(tc.tile_pool(name="const", bufs=1))
        w1pool = ctx.enter_context(tc.tile_pool(name="w1", bufs=3))
        hstp = ctx.enter_context(tc.tile_pool(name="hst", bufs=1))
        hqtp = ctx.enter_context(tc.tile_pool(name="hqt", bufs=1))
        tmpp = ctx.enter_context(tc.tile_pool(name="tmp", bufs=12))
        srowp = ctx.enter_context(tc.tile_pool(name="srow", bufs=3))
        epi = ctx.enter_context(tc.tile_pool(name="epi", bufs=1))
        ps_sc = ctx.enter_context(tc.tile_pool(name="ps_sc", bufs=QB, space="PSUM"))
        ps_h = ctx.enter_context(tc.tile_pool(name="ps_h", bufs=2, space="PSUM"))
        ps_q = ctx.enter_context(tc.tile_pool(name="ps_q", bufs=1, space="PSUM"))
        ps_ep = ctx.enter_context(tc.tile_pool(name="ps_ep", bufs=1, space="PSUM"))

        dma = nc.sync.dma_start

        misc = const.tile([128, MISC_W], f32, tag="misc")
        dma(misc[:], misc_d.ap())
        b1sb = misc[:, MISC_B1:MISC_B1 + MT]
        t1h = misc[0:QL, MISC_T1H:MISC_T1H + C]
        trev = misc[0:QL, MISC_TREV:MISC_TREV + 1]
        revi = misc[0:QL, MISC_REVI:MISC_REVI + C]
        ident = misc[0:QL, MISC_ID:MISC_ID + QL]
        mhot = [misc[:, MISC_MHOT + c4 * C: MISC_MHOT + (c4 + 1) * C] for c4 in range(4)]

        # preload the ACT Exp table off the critical path
        warm = const.tile([1, 2], f32, tag="warm")
        nc.scalar.activation(warm[0:1, 0:1], misc[0:1, 0:1], AF.Exp)

        for rep_i in range(reps):
            qtw2 = const.tile([128, QL * KT + MT], f32r, tag="qtw2")
            dma(qtw2[:], qtw2_d.ap())
            qt = [qtw2[:, QL * k:QL * (k + 1)] for k in range(KT)]
            w2sb = qtw2[:, QL * KT:QL * KT + MT]
            supt = []
            for k in range(KT):
                sup_k = const.tile([128, S], f32r, tag=f"sup{k}")
                supt.append(sup_k[:])
            dma(supt[0], supt_d.ap()[0])

            # chain step helper: tmp = relu(hst[m] + hqT col) on DVE/ACT, then
            # PE accumulates the W2-weighted reduction into the chain's PSUM row
            def chain_step(chains, qb, m, qi):
                q = qb + qi
                col = hqtb[m][:, q:q + 1]
                tmp = tmpp.tile([128, S], f32r, tag="tmp", name=f"tmp{qb}_{m}_{qi}")
                if TMP_ENG[(m * QB + qi) % 12] == "D":
                    nc.vector.tensor_scalar(
                        tmp[:], hst[m][:], col, 0.0, op0=OP.add, op1=OP.max
                    )
                else:
                    nc.scalar.activation(tmp[:], hst[m][:], AF.Relu, bias=col)
                nc.tensor.matmul(
                    chains[qi][0:1, :],
                    lhsT=w2sb[:, m:m + 1],
                    rhs=tmp[:],
                    start=(m == 0),
                    stop=(m == MT - 1),
                )

            def finish_block(chains, qb, bs):
                # stage the block's score rows and transpose into sct columns
                scrow = srowp.tile([QB, S], f32, tag="scrow", name=f"scrow{qb}")
                for qi in range(bs):
                    srow = srowp.tile([1, S], f32, tag="srow", name=f"srow{qb}_{qi}")
                    nc.scalar.copy(srow[:], chains[qi][0:1, :])
                    dma(scrow[qi:qi + 1, :], srow[:])
                for c4 in range(4):
                    nc.tensor.transpose(
                        sct_ps[:, c4 * QL + qb:c4 * QL + qb + bs],
                        scrow[0:bs, c4 * 128:(c4 + 1) * 128],
                        ident[0:bs, 0:bs],
                    )

            def open_block(bs):
                chains = []
                for qi in range(bs):
                    ch = ps_sc.tile([1, S], f32, tag="qs", name=f"qs{qi}")
                    chains.append(ch)
                return chains

            # ---- phase 1 + block-0 chains, pipelined with per-chunk W1T DMA ----
            epbank = ps_ep.tile([128, 128], f32, tag="epbank")
            sct_ps = epbank[:, 0:4 * QL]
            hqtb = []
            hst = []
            for m in range(MT):
                w1 = w1pool.tile([128, 2 * KT * 128], f32r, tag="w1")
                dma(w1[:], w1t_d.ap()[m])
                if m == 0:
                    for k in range(1, KT):
                        dma(supt[k], supt_d.ap()[k])

                ph = ps_h.tile([128, S], f32, tag="ph")
                for k in range(KT):
                    nc.tensor.matmul(
                        ph[:],
                        lhsT=w1[:, (KT + k) * 128:(KT + k + 1) * 128],
                        rhs=supt[k],
                        start=(k == 0),
                        stop=(k == KT - 1),
                    )
                hs_m = hstp.tile([128, S], f32, tag=f"hst{m}")
                nc.vector.tensor_copy(hs_m[:], ph[:])
                hst.append(hs_m)

                pq = ps_q.tile([128, QL], f32, tag="pq")
                for k in range(KT):
                    nc.tensor.matmul(
                        pq[:],
                        lhsT=w1[:, k * 128:(k + 1) * 128],
                        rhs=qt[k],
                        start=(k == 0),
                        stop=(k == KT - 1),
                    )
                hq_m = hqtp.tile([128, QL], f32, tag=f"hqtb{m}")
                nc.vector.tensor_scalar(
                    hq_m[:], pq[:], b1sb[:, m:m + 1], None, op0=OP.add
                )
                hqtb.append(hq_m)

            # ---- phase 2: query blocks ----
            for qb, bs in ((0, 4), (4, 4), (8, 4), (12, 2), (14, 2)):
                chains = open_block(bs)
                for m in range(MT):
                    for qi in range(bs):
                        chain_step(chains, qb, m, qi)
                finish_block(chains, qb, bs)

            # ---- epilogue ----
            sct_sb = epi.tile([128, 4 * QL], f32, tag="sct_sb")
            nc.vector.tensor_copy(sct_sb[:], sct_ps[:])

            agg_ps = epbank[0:QL, 4 * QL:4 * QL + C]
            for c4 in range(4):
                nc.tensor.matmul(
                    agg_ps[:],
                    lhsT=sct_sb[:, c4 * QL:(c4 + 1) * QL],
                    rhs=mhot[c4],
                    start=(c4 == 0),
                    stop=(c4 == 3),
                )
            agg_sb = epi.tile([QL, C], f32, tag="agg_sb")
            nc.vector.tensor_copy(agg_sb[:], agg_ps[:])

            # out4 columns: [ssum, mneg, zt, corr]; host computes
            # loss_q = log(ssum) - mneg - zt
            out4 = epi.tile([QL, 4], f32, tag="out4")

            mneg = out4[:, 1:2]
            nc.vector.tensor_reduce(mneg, agg_sb[:], mybir.AxisListType.X,
                                    OP.max, negate=True)
            mpos = epi.tile([QL, 1], f32, tag="mpos")
            nc.vector.tensor_scalar(mpos[:], mneg, -1.0, None, op0=OP.mult)

            ex = epi.tile([QL, C], f32, tag="ex")
            nc.scalar.activation(ex[:], agg_sb[:], AF.Exp, bias=mneg,
                                 accum_out=out4[:, 0:1])

            zt_t = epi.tile([QL, C], f32, tag="zt_t")
            nc.vector.tensor_tensor(zt_t[:], t1h, agg_sb[:], op=OP.mult)
            nc.vector.tensor_reduce(out4[:, 2:3], zt_t[:],
                                    mybir.AxisListType.X, OP.add)

            eq = epi.tile([QL, C], f32, tag="eq")
            nc.vector.tensor_scalar(eq[:], agg_sb[:], mpos[:, 0:1], None,
                                    op0=OP.is_equal)
            ev = epi.tile([QL, C], f32, tag="ev")
            nc.vector.tensor_tensor(ev[:], eq[:], revi, op=OP.mult)
            am = epi.tile([QL, 1], f32, tag="am")
            nc.vector.tensor_reduce(am[:], ev[:], mybir.AxisListType.X, OP.max)
            nc.vector.tensor_scalar(out4[:, 3:4], am[:], trev, None,
                                    op0=OP.is_equal)
            dma(out_d.ap(), out4[:])

    nc.compile()
    return nc


def _prep_inputs(query_reps, support_reps, W1, b1, W2, support_labels,
                 query_target_ids):
    """Host-side data layout: transposes, packing, one-hot metadata."""
    q = np.ascontiguousarray(np.asarray(query_reps, dtype=np.float32))
    sup = np.ascontiguousarray(np.asarray(support_reps, dtype=np.float32))
    W1 = np.asarray(W1, dtype=np.float32)
    b1 = np.asarray(b1, dtype=np.float32)
    W2 = np.asarray(W2, dtype=np.float32)
    labels = np.asarray(support_labels).astype(np.int64)
    tgt = np.asarray(query_target_ids).astype(np.int64)

    W1T = np.ascontiguousarray(W1.T)  # [2D, 2D]; W1T[d, d'] = W1[d', d]
    w1t = np.empty((MT, 128, 2 * KT * 128), np.float32)
    for m in range(MT):
        cols = W1T[:, m * 128:(m + 1) * 128]          # [2D, 128]
        for k in range(KT):
            w1t[m, :, k * 128:(k + 1) * 128] = cols[k * 128:(k + 1) * 128, :]
            w1t[m, :, (KT + k) * 128:(KT + k + 1) * 128] = \
                cols[D + k * 128:D + (k + 1) * 128, :]

    supT = sup.T  # [D, S]

    misc = np.zeros((128, MISC_W), np.float32)
    misc[:, MISC_B1:MISC_B1 + MT] = b1.reshape(MT, 128).T
    misc[0:QL, MISC_REVI:MISC_REVI + C] = np.broadcast_to(
        (C - 1 - np.arange(C)).astype(np.float32), (QL, C))
    misc[0:QL, MISC_ID:MISC_ID + QL] = np.eye(QL, dtype=np.float32)
    cnt = np.bincount(labels, minlength=C).astype(np.float32)
    mhot = np.zeros((S, C), np.float32)
    mhot[np.arange(S), labels] = (1.0 / np.maximum(cnt, 1.0))[labels]
    for c4 in range(4):
        misc[:, MISC_MHOT + c4 * C:MISC_MHOT + (c4 + 1) * C] = \
            mhot[c4 * 128:(c4 + 1) * 128]

    supt_in = np.ascontiguousarray(supT.reshape(KT, 128, S))

    in_maps = []
    for i in range(NCORES):
        qi = q[i * QL:(i + 1) * QL]                  # [QL, D]
        qT = qi.T                                    # [D, QL]
        qtw2 = np.zeros((128, QL * KT + MT), np.float32)
        for k in range(KT):
            qtw2[:, QL * k:QL * (k + 1)] = qT[k * 128:(k + 1) * 128]
        qtw2[:, QL * KT:QL * KT + MT] = W2[0].reshape(MT, 128).T
        ti = tgt[i * QL:(i + 1) * QL]
        mi = misc.copy()
        t1h = np.zeros((QL, C), np.float32)
        t1h[np.arange(QL), ti] = 1.0
        mi[0:QL, MISC_T1H:MISC_T1H + C] = t1h
        mi[0:QL, MISC_TREV:MISC_TREV + 1] = \
            (C - 1 - ti).astype(np.float32).reshape(QL, 1)
        in_maps.append({"w1t": w1t, "qtw2": qtw2, "supt": supt_in, "misc": mi})
    return in_maps


def get_program(reps=1):
    if reps not in _CACHE:
        _CACHE[reps] = _build_program(reps=reps)
    return _CACHE[reps]


def run_on_device(nc, in_maps):
    from concourse.bass_utils import run_bass_kernel_spmd
    return run_bass_kernel_spmd(nc, in_maps, core_ids=list(range(NCORES)),
                                trace=False)


def kernel(query_reps, support_reps, W1, b1, W2, b2, support_labels,
           query_target_ids, num_classes):
    global LAST_EXEC_NS
    assert int(num_classes) == C

    nc = get_program(reps=1)
    in_maps = _prep_inputs(query_reps, support_reps, W1, b1, W2,
                           support_labels, query_target_ids)
    res = run_on_device(nc, in_maps)
    LAST_EXEC_NS = res.exec_time_ns

    out = np.concatenate([res.results[i]["out4"] for i in range(NCORES)], axis=0)
    ssum, mneg, zt, corrf = out[:, 0], out[:, 1], out[:, 2], out[:, 3]
    loss_all = np.log(ssum) - mneg - zt
    loss = np.float32(np.mean(loss_all))
    correct = corrf > 0.5
    return loss, correct


# revision 38
# speedup vs baseline: 845.8748x; 1.0439x over previous
"""Trainium2 Bass kernel for nn_CBERTProtoConcat (retrieval_knn).

Computation (reference):
    hq = query @ W1[:, :D].T            [Q, 2D]
    hs = support @ W1[:, D:].T          [S, 2D]
    scores[q,s] = sum_d W2[d] * relu(hq[q,d] + hs[s,d] + b1[d])    [Q, S]
    agg = segment-mean of scores by support label                  [Q, C]
    loss = -mean(log_softmax(agg)[q, target_q]);  correct = argmax(agg) == target

Strategy: shard Q across 8 cores (16 queries each), replicate everything else
(collective-free).  Per core, layout: d' (=2D) on partitions, s on free.
  - hqT [2D, 16] and hsT [2D, S] via PE matmuls (f32r @ 1 cyc/row), pipelined
    against the per-chunk W1T DMA stream.
  - per-query reduction chains, in blocks of <=4 (PSUM bank limited): DVE/ACT
    compute tmp = relu(hsT_m + hqT[:,q]) [128, 512] (dual-op tensor_scalar on
    DVE / biased Relu activation on ACT, ~2:1 split), and PE accumulates
    scores_q[1, 512] += W2_m.T @ tmp in PSUM over the 12 m-steps (f32r).
  - per-block: score rows staged via partition-0 SBUF + DMA, transposed by PE
    into scoresT columns; then one-hot matmul -> agg [16, C] (count-normalized
    one-hot built on host from the labels), exp/sums on device.
  - outputs per core: [softmax-sum, -max, target-logit, correct] x 16; the
    host finishes with log(ssum) and the mean over 128 queries.
b2 is skipped: it shifts all logits uniformly (log_softmax/argmax invariant).
Cost-model (TimelineSim) estimate: ~88 us end-to-end per core; engine busy:
PE 62, DVE 53, ACT 51 us.
"""

import numpy as np

NCORES = 8
Q, S, D = 128, 512, 768
D2 = 2 * D            # 1536
C = 64                # num classes
QL = Q // NCORES      # 16 queries per core
KT = D // 128         # 6 contraction tiles per slice
MT = D2 // 128        # 12 d' tiles
QB = 4                # query block size in phase 2

# engine assignment pattern for tmp production (7 DVE, 3 ACT, 2 Pool per 12)
TMP_ENG = ["D", "D", "A", "D", "D", "A", "D", "D", "A", "D", "D", "A"]

# packed input layouts
QSW_QT0 = 0                      # qt_k at [.., 16k:16k+16]
QSW_SUP0 = QL * KT               # 96;  supt_k at [96+512k : 96+512(k+1)]
QSW_W2 = QSW_SUP0 + S * KT       # 3168; w2 cols [3168:3180]
QSW_W = QSW_W2 + MT              # 3180

MISC_B1 = 0                      # [128, 0:12]
MISC_T1H = MT                    # [0:16, 12:76]
MISC_TREV = MISC_T1H + C         # [0:16, 76:77]
MISC_REVI = MISC_TREV + 1        # [0:16, 77:141]
MISC_ID = MISC_REVI + C          # [0:16, 141:157]
MISC_MHOT = MISC_ID + QL         # [128, 157:157+256]
MISC_W = MISC_MHOT + 4 * C       # 413

LAST_EXEC_NS = None

_CACHE = {}


def _build_program(reps=1):
    from contextlib import ExitStack

    import concourse.tile as tile
    from concourse import bacc, mybir

    f32 = mybir.dt.float32
    f32r = mybir.dt.float32r
    AF = mybir.ActivationFunctionType
    OP = mybir.AluOpType

    nc = bacc.Bacc(
        "TRN2",
        target_bir_lowering=False,
        debug=False,
        enable_asserts=False,
        num_devices=NCORES,
    )

    # ---- I/O ----
    # w1t[m] is [128, KT*2*128]: slice1 chunk k at [:, 128k:...], slice2 at [:, (KT+k)*128:...]
    w1t_d = nc.dram_tensor("w1t", [MT, 128, 2 * KT * 128], f32r, kind="ExternalInput")
    qtw2_d = nc.dram_tensor("qtw2", [128, QL * KT + MT], f32r, kind="ExternalInput")
    supt_d = nc.dram_tensor("supt", [KT, 128, S], f32r, kind="ExternalInput")
    misc_d = nc.dram_tensor("misc", [128, MISC_W], f32, kind="ExternalInput")

    out_d = nc.dram_tensor("out4", [QL, 4], f32, kind="ExternalOutput")

    with tile.TileContext(nc) as tc, ExitStack() as ctx:
        const = ctx.enter_context(tc.tile_pool(name="const", bufs=1))
        w1pool = ctx.enter_context(tc.tile_pool(name="w1", bufs=3))
        hstp = ctx.enter_context(tc.tile_pool(name="hst", bufs=1))
        hqtp = ctx.enter_context(tc.tile_pool(name="hqt", bufs=1))
        tmpp = ctx.enter_context(tc.tile_pool(name="tmp", bufs=40))
        srowp = ctx.enter_context(tc.tile_pool(name="srow", bufs=3))
        epi = ctx.enter_context(tc.tile_pool(name="epi", bufs=1))
        ps_sc = ctx.enter_context(tc.tile_pool(name="ps_sc", bufs=1, space="PSUM"))
        ps_h = ctx.enter_context(tc.tile_pool(name="ps_h", bufs=2, space="PSUM"))
        ps_q = ctx.enter_context(tc.tile_pool(name="ps_q", bufs=1, space="PSUM"))
        ps_ep = ctx.enter_context(tc.tile_pool(name="ps_ep", bufs=1, space="PSUM"))

        dma = nc.sync.dma_start

        misc = const.tile([128, MISC_W], f32, tag="misc")
        dma(misc[:], misc_d.ap())
        b1sb = misc[:, MISC_B1:MISC_B1 + MT]
        t1h = misc[0:QL, MISC_T1H:MISC_T1H + C]
        trev = misc[0:QL, MISC_TREV:MISC_TREV + 1]
        revi = misc[0:QL, MISC_REVI:MISC_REVI + C]
        ident = misc[0:QL, MISC_ID:MISC_ID + QL]
        mhot = [misc[:, MISC_MHOT + c4 * C: MISC_MHOT + (c4 + 1) * C] for c4 in range(4)]

        # preload the ACT Exp table off the critical path
        warm = const.tile([1, 2], f32, tag="warm")
        nc.scalar.activation(warm[0:1, 0:1], misc[0:1, 0:1], AF.Exp)

        for rep_i in range(reps):
            w1_first = w1pool.tile([128, 2 * KT * 128], f32r, tag="w1", name="w1_first")
            dma(w1_first[:], w1t_d.ap()[0])
            qtw2 = const.tile([128, QL * KT + MT], f32r, tag="qtw2")
            dma(qtw2[:], qtw2_d.ap())
            qt = [qtw2[:, QL * k:QL * (k + 1)] for k in range(KT)]
            w2sb = qtw2[:, QL * KT:QL * KT + MT]
            supt = []
            for k in range(KT):
                sup_k = const.tile([128, S], f32r, tag=f"sup{k}")
                supt.append(sup_k[:])


            # chain step helper: tmp = relu(hst[m] + hqT col) on DVE/ACT, then
            # PE accumulates the W2-weighted reduction into the chain's PSUM row
            def chain_step(chains, qb, m, qi):
                q = qb + qi
                col = hqtb[m][:, q:q + 1]
                tmp = tmpp.tile([128, S], f32r, tag="tmp", name=f"tmp{qb}_{m}_{qi}")
                if TMP_ENG[(m * 4 + qi) % 12] == "D":
                    nc.vector.tensor_scalar(
                        tmp[:], hst[m][:], col, 0.0, op0=OP.add, op1=OP.max
                    )
                else:
                    nc.scalar.activation(tmp[:], hst[m][:], AF.Relu, bias=col)
                nc.tensor.matmul(
                    chains[qi],
                    lhsT=w2sb[:, m:m + 1],
                    rhs=tmp[:],
                    start=(m == 0),
                    stop=(m == MT - 1),
                )

            def finish_block(chains, blk, qb, bs):
                # one PSUM->SBUF copy for the whole block, one DMA to split the
                # rows across partitions, then PE transposes into sct columns
                srow = srowp.tile([1, QB * S], f32, tag="srow", name=f"srow{qb}")
                half = (bs * S) // 2
                nc.scalar.copy(srow[0:1, 0:half], blk[0:1, 0:half])
                nc.vector.tensor_copy(srow[0:1, half:bs * S], blk[0:1, half:bs * S])
                scrow = srowp.tile([QB, S], f32, tag="scrow", name=f"scrow{qb}")
                dma(scrow[0:bs, :],
                    srow[0:1, 0:bs * S].rearrange("p (b s) -> (p b) s", s=S))
                for c4 in range(4):
                    nc.tensor.transpose(
                        sct_ps[:, c4 * QL + qb:c4 * QL + qb + bs],
                        scrow[0:bs, c4 * 128:(c4 + 1) * 128],
                        ident[0:bs, 0:bs],
                    )

            def open_block(bs):
                blk = ps_sc.tile([1, QB * S], f32, tag="qsblk", name="qsblk")
                chains = [blk[0:1, qi * S:(qi + 1) * S] for qi in range(bs)]
                return blk, chains

            # ---- phase 1 + block-0 chains, pipelined with per-chunk W1T DMA ----
            epbank = ps_ep.tile([128, 128], f32, tag="epbank")
            sct_ps = epbank[:, 0:4 * QL]
            hqtb = []
            hst = []
            for m in range(MT):
                if m == 0:
                    w1 = w1_first
                    for k in range(KT):
                        dma(supt[k], supt_d.ap()[k])
                else:
                    w1 = w1pool.tile([128, 2 * KT * 128], f32r, tag="w1")
                    dma(w1[:], w1t_d.ap()[m])

                pq = ps_q.tile([128, QL], f32, tag="pq")
                for k in range(KT):
                    nc.tensor.matmul(
                        pq[:],
                        lhsT=w1[:, k * 128:(k + 1) * 128],
                        rhs=qt[k],
                        start=(k == 0),
                        stop=(k == KT - 1),
                    )
                hq_m = hqtp.tile([128, QL], f32, tag=f"hqtb{m}")
                nc.vector.tensor_scalar(
                    hq_m[:], pq[:], b1sb[:, m:m + 1], None, op0=OP.add
                )
                hqtb.append(hq_m)

                ph = ps_h.tile([128, S], f32, tag="ph")
                for k in range(KT):
                    nc.tensor.matmul(
                        ph[:],
                        lhsT=w1[:, (KT + k) * 128:(KT + k + 1) * 128],
                        rhs=supt[k],
                        start=(k == 0),
                        stop=(k == KT - 1),
                    )
                hs_m = hstp.tile([128, S], f32, tag=f"hst{m}")
                if m % 2 == 0:
                    nc.vector.tensor_copy(hs_m[:], ph[:])
                else:
                    nc.scalar.copy(hs_m[:], ph[:])
                hst.append(hs_m)

            # ---- phase 2: query blocks ----
            for qb, bs in tuple((b, QB) for b in range(0, QL, QB)):
                blk, chains = open_block(bs)
                for m in range(MT):
                    for qi in range(bs):
                        chain_step(chains, qb, m, qi)
                finish_block(chains, blk, qb, bs)

            # ---- epilogue ----
            sct_sb = epi.tile([128, 4 * QL], f32, tag="sct_sb")
            nc.vector.tensor_copy(sct_sb[:], sct_ps[:])

            agg_ps = epbank[0:QL, 4 * QL:4 * QL + C]
            for c4 in range(4):
                nc.tensor.matmul(
                    agg_ps[:],
                    lhsT=sct_sb[:, c4 * QL:(c4 + 1) * QL],
                    rhs=mhot[c4],
                    start=(c4 == 0),
                    stop=(c4 == 3),
                )
            agg_sb = epi.tile([QL, C], f32, tag="agg_sb")
            nc.vector.tensor_copy(agg_sb[:], agg_ps[:])

            # out4 columns: [ssum, mneg, zt, corr]; host computes
            # loss_q = log(ssum) - mneg - zt
            out4 = epi.tile([QL, 4], f32, tag="out4")

            mneg = out4[:, 1:2]
            nc.vector.tensor_reduce(mneg, agg_sb[:], mybir.AxisListType.X,
                                    OP.max, negate=True)
            mpos = epi.tile([QL, 1], f32, tag="mpos")
            nc.vector.tensor_scalar(mpos[:], mneg, -1.0, None, op0=OP.mult)

            ex = epi.tile([QL, C], f32, tag="ex")
            nc.scalar.activation(ex[:], agg_sb[:], AF.Exp, bias=mneg,
                                 accum_out=out4[:, 0:1])

            zt_t = epi.tile([QL, C], f32, tag="zt_t")
            nc.vector.tensor_tensor(zt_t[:], t1h, agg_sb[:], op=OP.mult)
            nc.vector.tensor_reduce(out4[:, 2:3], zt_t[:],
                                    mybir.AxisListType.X, OP.add)

            eq = epi.tile([QL, C], f32, tag="eq")
            nc.vector.tensor_scalar(eq[:], agg_sb[:], mpos[:, 0:1], None,
                                    op0=OP.is_equal)
            ev = epi.tile([QL, C], f32, tag="ev")
            nc.vector.tensor_tensor(ev[:], eq[:], revi, op=OP.mult)
            am = epi.tile([QL, 1], f32, tag="am")
            nc.vector.tensor_reduce(am[:], ev[:], mybir.AxisListType.X, OP.max)
            nc.vector.tensor_scalar(out4[:, 3:4], am[:], trev, None,
                                    op0=OP.is_equal)
            dma(out_d.ap(), out4[:])

    nc.compile()
    return nc


def _prep_inputs(query_reps, support_reps, W1, b1, W2, support_labels,
                 query_target_ids):
    """Host-side data layout: transposes, packing, one-hot metadata."""
    q = np.ascontiguousarray(np.asarray(query_reps, dtype=np.float32))
    sup = np.ascontiguousarray(np.asarray(support_reps, dtype=np.float32))
    W1 = np.asarray(W1, dtype=np.float32)
    b1 = np.asarray(b1, dtype=np.float32)
    W2 = np.asarray(W2, dtype=np.float32)
    labels = np.asarray(support_labels).astype(np.int64)
    tgt = np.asarray(query_target_ids).astype(np.int64)

    W1T = np.ascontiguousarray(W1.T)  # [2D, 2D]; W1T[d, d'] = W1[d', d]
    w1t = np.empty((MT, 128, 2 * KT * 128), np.float32)
    for m in range(MT):
        cols = W1T[:, m * 128:(m + 1) * 128]          # [2D, 128]
        for k in range(KT):
            w1t[m, :, k * 128:(k + 1) * 128] = cols[k * 128:(k + 1) * 128, :]
            w1t[m, :, (KT + k) * 128:(KT + k + 1) * 128] = \
                cols[D + k * 128:D + (k + 1) * 128, :]

    supT = sup.T  # [D, S]

    misc = np.zeros((128, MISC_W), np.float32)
    misc[:, MISC_B1:MISC_B1 + MT] = b1.reshape(MT, 128).T
    misc[0:QL, MISC_REVI:MISC_REVI + C] = np.broadcast_to(
        (C - 1 - np.arange(C)).astype(np.float32), (QL, C))
    misc[0:QL, MISC_ID:MISC_ID + QL] = np.eye(QL, dtype=np.float32)
    cnt = np.bincount(labels, minlength=C).astype(np.float32)
    mhot = np.zeros((S, C), np.float32)
    mhot[np.arange(S), labels] = (1.0 / np.maximum(cnt, 1.0))[labels]
    for c4 in range(4):
        misc[:, MISC_MHOT + c4 * C:MISC_MHOT + (c4 + 1) * C] = \
            mhot[c4 * 128:(c4 + 1) * 128]

    supt_in = np.ascontiguousarray(supT.reshape(KT, 128, S))

    in_maps = []
    for i in range(NCORES):
        qi = q[i * QL:(i + 1) * QL]                  # [QL, D]
        qT = qi.T                                    # [D, QL]
        qtw2 = np.zeros((128, QL * KT + MT), np.float32)
        for k in range(KT):
            qtw2[:, QL * k:QL * (k + 1)] = qT[k * 128:(k + 1) * 128]
        qtw2[:, QL * KT:QL * KT + MT] = W2[0].reshape(MT, 128).T
        ti = tgt[i * QL:(i + 1) * QL]
        mi = misc.copy()
        t1h = np.zeros((QL, C), np.float32)
        t1h[np.arange(QL), ti] = 1.0
        mi[0:QL, MISC_T1H:MISC_T1H + C] = t1h
        mi[0:QL, MISC_TREV:MISC_TREV + 1] = \
            (C - 1 - ti).astype(np.float32).reshape(QL, 1)
        in_maps.append({"w1t": w1t, "qtw2": qtw2, "supt": supt_in, "misc": mi})
    return in_maps


def get_program(reps=1):
    if reps not in _CACHE:
        _CACHE[reps] = _build_program(reps=reps)
    return _CACHE[reps]


def run_on_device(nc, in_maps):
    from concourse.bass_utils import run_bass_kernel_spmd
    return run_bass_kernel_spmd(nc, in_maps, core_ids=list(range(NCORES)),
                                trace=False)


def kernel(query_reps, support_reps, W1, b1, W2, b2, support_labels,
           query_target_ids, num_classes):
    global LAST_EXEC_NS
    assert int(num_classes) == C

    nc = get_program(reps=1)
    in_maps = _prep_inputs(query_reps, support_reps, W1, b1, W2,
                           support_labels, query_target_ids)
    res = run_on_device(nc, in_maps)
    LAST_EXEC_NS = res.exec_time_ns

    out = np.concatenate([res.results[i]["out4"] for i in range(NCORES)], axis=0)
    ssum, mneg, zt, corrf = out[:, 0], out[:, 1], out[:, 2], out[:, 3]
    loss_all = np.log(ssum) - mneg - zt
    loss = np.float32(np.mean(loss_all))
    correct = corrf > 0.5
    return loss, correct
